# revision 3
# baseline (speedup 1.0000x reference)
"""Expert-parallel MoE feed-forward (top-2 routing) on 8 TRN2 NeuronCores.

Strategy: one expert per core (E == n_cores == 8). Token routing is part of
input sharding: host gathers each expert's assigned token activations
(transposed, bf16) and feeds core e only its tokens plus its expert's three
weight matrices. Each core runs a dense FFN
    out = (silu(x @ Wg^T) * (x @ Wu^T)) @ Wd^T
over its token batch in bf16 (fp32 PSUM accumulation), entirely from SBUF.
Host scatters per-core outputs back into the (T, A, D) result.
"""

import math
import sys
import types

import numpy as np
import ml_dtypes

T, D, H, E, A = 4096, 1024, 2048, 8, 2
N_CORES = 8
BF16 = ml_dtypes.bfloat16

# Filled by kernel() with the BassKernelResults of the last device run so an
# external harness (test.py) can read exec_time_ns when tracing is on.
LAST_RESULT = None

_SHIMS_DONE = False


def _install_shims():
    """Environment fixes for running Bass/Tile SPMD kernels under axon."""
    global _SHIMS_DONE
    if _SHIMS_DONE:
        return
    _SHIMS_DONE = True

    # 1. NTFF profile hook (lets trace=True / BASS_TRACE=1 report exec_time_ns).
    if "antenv.axon_hooks" not in sys.modules:
        try:
            import antenv.axon_hooks  # noqa: F401  (real module present)
        except ImportError:
            _hook = None
            try:
                import trn_agent_boot.trn_boot as tb

                _hook = tb._ntff_profile_via_ctypes("/opt/axon/libaxon_pjrt.so")
            except Exception:
                _hook = None
            mod = types.ModuleType("antenv.axon_hooks")
            mod.get_axon_ntff_profile_hook = lambda: _hook
            sys.modules["antenv.axon_hooks"] = mod

    # 2. No artifact upload from a zero-egress container.
    from concourse import bass_utils

    bass_utils.upload_artifacts = lambda tmpdir: f"local:{tmpdir}"

    # 3. This walrus build allows only one sync-wait command on a CTRL
    # (Drain) instruction; split the tile-exit drain's waits onto nops.
    import concourse.tile as tile
    from concourse import mybir
    from concourse.vector_clock import ScopedClock

    if getattr(tile.TileContext._drain_and_barrier, "_is_patched", False):
        return

    def _patched_drain_and_barrier(self, tick_clock, wait_clock):
        nc = self.nc
        drain_inst = nc.sync.drain()
        wait_clock.add_sem_waits(
            drain_inst.ins, ScopedClock({None: tick_clock.global_clock})
        )
        ow = drain_inst.ins.sync_info.on_wait if drain_inst.ins.sync_info else None
        maxw = 1
        if ow and len(ow) > maxw:
            extra = list(ow[maxw:])
            del ow[maxw:]
            for i in range(0, len(extra), maxw):
                nop = nc.sync.nop(hint="drain_split", nofuse=True)
                if nop.ins.sync_info is None:
                    nop.ins.sync_info = mybir.SyncInfo(on_wait=[], on_update=[])
                for w in extra[i : i + maxw]:
                    nop.ins.sync_info.on_wait.append(w)
        nc.all_engine_barrier()
        assert self.sems is not None
        popped = nc._tile_sem_poison_stack.pop()
        assert popped is self._sem_poison
        nc.clear_and_free_semaphores(list(self.sems.allocated().values()))
        nc.all_engine_barrier()

    _patched_drain_and_barrier._is_patched = True
    tile.TileContext._drain_and_barrier = _patched_drain_and_barrier


def _split_multi_waits(nc):
    """This walrus build allows one sync-wait command per instruction.

    Tile's sem assignment can attach several; move the extras onto nofuse
    NoOps inserted just before the instruction on the same engine (engines
    execute a block's instructions in order, so semantics are unchanged).
    """
    import bass_rust
    from concourse import mybir

    ctr = 0
    for f in nc.m.functions:
        for bb in f.blocks:
            new = []
            changed = False
            for inst in bb.instructions:
                si = inst.sync_info
                ow = si.on_wait if si else None
                if ow is not None and len(ow) > 1:
                    extra = list(ow[:-1])
                    del ow[:-1]
                    for w in extra:
                        ctr += 1
                        nop = bass_rust.InstNoOp()
                        nop.name = f"I-wsplit-{ctr}"
                        nop.engine = inst.engine
                        nop.sync_info = mybir.SyncInfo(on_wait=[w], on_update=[])
                        nop.bass_nofuse = True
                        new.append(nop)
                    changed = True
                new.append(inst)
            if changed:
                bb.instructions = new


def _chunk_sizes(cap):
    """Split cap token columns into near-equal chunks of <=512 (PSUM bank)."""
    n = max(1, math.ceil(cap / 512))
    base = cap // n
    rem = cap - base * n
    return [base + (1 if i < rem else 0) for i in range(n)]


def _build_nc(cap):
    import concourse.bass as bass
    import concourse.tile as tile
    from concourse import mybir

    f32 = mybir.dt.float32
    bf16 = mybir.dt.bfloat16
    KD = D // 128  # 8  k-tiles over the model dim
    KH = H // 128  # 16 k-tiles over the hidden dim
    chunks = _chunk_sizes(cap)
    cmax = max(chunks)

    nc = bass.Bass()
    xT = nc.dram_tensor("xT", [D, cap], bf16, kind="ExternalInput")
    wgT = nc.dram_tensor("wgT", [D, H], bf16, kind="ExternalInput")
    wuT = nc.dram_tensor("wuT", [D, H], bf16, kind="ExternalInput")
    wdT = nc.dram_tensor("wdT", [H, D], bf16, kind="ExternalInput")
    out = nc.dram_tensor("out", [D, cap], bf16, kind="ExternalOutput")

    with tile.TileContext(nc) as tc:
        with (
            tc.tile_pool(name="wpool", bufs=1) as wpool,
            tc.tile_pool(name="hpool", bufs=2) as hpool,
            tc.tile_pool(name="spool", bufs=4) as spool,
            tc.tile_pool(name="opool", bufs=4) as opool,
            tc.tile_pool(name="psum", bufs=2, space="PSUM") as psum,
        ):
            # Inputs first: x, then gate weights (needed first), then up, down.
            x_sb = []
            for ki in range(KD):
                t = wpool.tile([128, cap], bf16, tag=f"x{ki}", name=f"x_sb{ki}")
                nc.sync.dma_start(t[:], xT[128 * ki : 128 * (ki + 1), :])
                x_sb.append(t)
            wg_sb = []
            for ki in range(KD):
                t = wpool.tile([128, H], bf16, tag=f"wg{ki}", name=f"wg_sb{ki}")
                nc.sync.dma_start(t[:], wgT[128 * ki : 128 * (ki + 1), :])
                wg_sb.append(t)
            wu_sb = []
            for ki in range(KD):
                t = wpool.tile([128, H], bf16, tag=f"wu{ki}", name=f"wu_sb{ki}")
                nc.sync.dma_start(t[:], wuT[128 * ki : 128 * (ki + 1), :])
                wu_sb.append(t)
            wd_sb = []
            for hk in range(KH):
                t = wpool.tile([128, D], bf16, tag=f"wd{hk}", name=f"wd_sb{hk}")
                nc.sync.dma_start(t[:], wdT[128 * hk : 128 * (hk + 1), :])
                wd_sb.append(t)

            c0 = 0
            for cn in chunks:
                csl = slice(c0, c0 + cn)
                h_sb = hpool.tile([128, KH * cmax], bf16, tag="h", name="h_sb")
                for hi in range(KH):
                    hsl = slice(128 * hi, 128 * (hi + 1))
                    pg = psum.tile([128, 512], f32, tag="pg", name="pg")
                    for ki in range(KD):
                        nc.tensor.matmul(
                            pg[:, :cn],
                            wg_sb[ki][:, hsl],
                            x_sb[ki][:, csl],
                            start=(ki == 0),
                            stop=(ki == KD - 1),
                        )
                    pu = psum.tile([128, 512], f32, tag="pu", name="pu")
                    for ki in range(KD):
                        nc.tensor.matmul(
                            pu[:, :cn],
                            wu_sb[ki][:, hsl],
                            x_sb[ki][:, csl],
                            start=(ki == 0),
                            stop=(ki == KD - 1),
                        )
                    sg = spool.tile([128, 512], bf16, tag="s", name="sg")
                    nc.scalar.activation(
                        sg[:, :cn], pg[:, :cn], mybir.ActivationFunctionType.Silu
                    )
                    nc.vector.tensor_mul(
                        h_sb[:, cmax * hi : cmax * hi + cn], sg[:, :cn], pu[:, :cn]
                    )
                for di in range(KD):
                    dsl = slice(128 * di, 128 * (di + 1))
                    po = psum.tile([128, 512], f32, tag="po", name="po")
                    for hk in range(KH):
                        nc.tensor.matmul(
                            po[:, :cn],
                            wd_sb[hk][:, dsl],
                            h_sb[:, cmax * hk : cmax * hk + cn],
                            start=(hk == 0),
                            stop=(hk == KH - 1),
                        )
                    o = opool.tile([128, 512], bf16, tag="o", name="o")
                    nc.vector.tensor_copy(o[:, :cn], po[:, :cn])
                    nc.sync.dma_start(out[dsl, csl], o[:, :cn])
                c0 += cn
    _split_multi_waits(nc)
    return nc


def kernel(x, expert_indices, w_gate, w_up, w_down):
    global LAST_RESULT
    _install_shims()
    from concourse import bass_utils

    x = np.asarray(x)
    ei = np.asarray(expert_indices).astype(np.int64)
    w_gate = np.asarray(w_gate)
    w_up = np.asarray(w_up)
    w_down = np.asarray(w_down)

    flat = ei.reshape(-1)  # pair p = t*A + a  ->  expert id
    counts = np.bincount(flat, minlength=E)
    order = np.argsort(flat, kind="stable")
    starts = np.zeros(E + 1, dtype=np.int64)
    np.cumsum(counts, out=starts[1:])
    cap = int(counts.max())
    cap = max(cap, 128)

    idx_per_core = []
    in_maps = []
    for e in range(E):
        idx = order[starts[e] : starts[e + 1]]
        idx_per_core.append(idx)
        tok = idx // A
        xeT = np.zeros((D, cap), dtype=BF16)
        xeT[:, : len(idx)] = x[tok].T.astype(BF16)
        in_maps.append(
            {
                "xT": xeT,
                "wgT": np.ascontiguousarray(w_gate[e].T).astype(BF16),
                "wuT": np.ascontiguousarray(w_up[e].T).astype(BF16),
                "wdT": np.ascontiguousarray(w_down[e].T).astype(BF16),
            }
        )

    nc = _build_nc(cap)
    res = bass_utils.run_bass_kernel_spmd(nc, in_maps, core_ids=list(range(N_CORES)))
    LAST_RESULT = res

    out = np.zeros((T * A, D), dtype=np.float32)
    for e in range(E):
        idx = idx_per_core[e]
        oT = np.asarray(res.results[e]["out"])  # [D, cap] bf16
        out[idx] = oT[:, : len(idx)].T.astype(np.float32)
    return out.reshape(T, A, D)
